# revision 33
# baseline (speedup 1.0000x reference)
"""Single-head causal attention (B=8, T=2048, C=1024, H=128) on 8 TRN2 NeuronCores.

Strategy: pure data-parallel over batch — one batch element per core, zero
collectives.  v2 schedule: keep the PE continuously busy from the framework
start barrier (~6.8us) to the last PV chain:

  - warmup filler matmuls on a memset tile start at the barrier so the HAM
    clock-gate releases (1.2 -> 2.4 GHz) ~3.4us in, and the PE has work
    during the input-DMA latency window.
  - input DMAs are depth-limited (tiny same-engine stall reads) so each DMA
    ring finishes tiles in demand order instead of round-robining them all
    (RR makes the first tile land as late as the last).  hh=0 tiles split
    across the scalar+sync HW-DGE rings; hh=1 tiles whole on the gpsimd
    SW-DGE ring (its ~3us startup is hidden; they're needed late).
  - projection emitted per 512-col segment (cc-inner); q/k/v drains all on
    DVE (drain order k,q,v) so ACT does nothing but exp; S(j) units weave
    into the next projection segment's matmuls at ~2 S units per 2 chunk
    triples so exp (ACT) streams while the PE keeps projecting.
  - S units are single k-tiles [128(k), 512(q)] in one PSUM bank: matmul
    only the causal column range, exp only that range, triangle mask on
    GpSimd for diagonal tiles.  PSUM: 3 proj-acc banks (reused by PV accum
    tiles at the tail) + 4 S banks + 1 warmup/transpose bank = 8.
  - all PV chains run at the tail, woven with S3 units so PV0-2 fill the
    PE while ACT streams S3's exps; PV3 (the only exp-gated work) starts
    right as its exps complete.  out_aug ones-column gives the softmax
    denominator; DVE reciprocal+scale; one output DMA per 128-row q tile.
"""

import numpy as np
import ml_dtypes

import concourse.bass as bass  # noqa: F401
import concourse.mybir as mybir
import concourse.tile as tile
from concourse import bacc
from concourse.bass_utils import run_bass_kernel_spmd

B, T, C, H = 8, 2048, 1024, 128
NCORES = 8
P = 128
SEG = 512
BF16 = mybir.dt.bfloat16
F32 = mybir.dt.float32
SCALE = float(C) ** -0.5

NFILL = 40  # warmup filler matmuls (cover barrier -> first-input latency)

LAST_RESULT = None


def build_nc(t=T, reps=1):
    nchunk = C // P      # 8
    ntile = t // P       # 16 k-tiles
    nblk = t // SEG      # 4 q-blocks
    tpb = SEG // P       # 4 q-tiles per block
    nhalf = max(1, t // 1024)
    hw = t // nhalf      # 1024

    nc = bacc.Bacc("TRN2", target_bir_lowering=False, debug=False)

    npair = (C // P) // 2
    nseg = hw // SEG  # 2 segments per half
    xt_d = nc.dram_tensor("xt", [2, nseg, npair, P, 2, SEG], BF16,
                          kind="ExternalInput")
    w3_d = nc.dram_tensor("w3", [P, nchunk, 3, H], BF16, kind="ExternalInput")
    b3_d = nc.dram_tensor("b3", [H, 3], F32, kind="ExternalInput")
    te_d = nc.dram_tensor("te", [P, 2 * P], BF16, kind="ExternalInput")
    out_d = nc.dram_tensor("out", [t, H], F32, kind="ExternalOutput")

    Exp = mybir.ActivationFunctionType.Exp
    n_es = (nblk * (nblk + 1) // 2) * tpb  # total S units

    with tile.TileContext(nc) as tc:
        with (
            tc.tile_pool(name="const", bufs=1) as cpool,
            tc.tile_pool(name="big", bufs=1) as bpool,
            tc.tile_pool(name="v", bufs=ntile) as vpool,
            tc.tile_pool(name="es", bufs=n_es) as espool,
            tc.tile_pool(name="o", bufs=1) as opool,
            tc.tile_pool(name="ps", bufs=1, space="PSUM") as pspool,
        ):
          for rep in range(reps):
            # ---- PE warmup fillers (deps: one DVE memset only)
            warm_s = cpool.tile([P, SEG], BF16, tag="warm", name="warm_t")
            nc.vector.memset(warm_s[:], 0.0)
            warm_ps = pspool.tile([P, 2, H + 32], F32, tag="pso", bufs=2,
                                  name="warm_ps")
            for _ in range(NFILL):
                nc.tensor.matmul(warm_ps[:, 0, 0:P], warm_s[:, 0:P],
                                 warm_s[:, P:2 * P], start=True, stop=True)

            # ---- input tiles (one per pair/half/segment, 256KB each)
            xt_s = {(pp, hh, ss): cpool.tile([P, 2, SEG], BF16,
                                             tag=f"xtp{pp}_{hh}_{ss}",
                                             name=f"xtp{pp}_{hh}_{ss}")
                    for pp in range(npair) for hh in range(2)
                    for ss in range(nseg)}
            w_s = cpool.tile([P, nchunk, 3, H], BF16, tag="w3", name="w3_t")
            te_s = cpool.tile([P, 2 * P], BF16, tag="te", name="te_t")
            b_s = cpool.tile([P, 3], F32, tag="b3", name="b3_t")
            fl_s = cpool.tile([P, 8], BF16, tag="flow", name="flow_t")
            tri_s = te_s[:, 0:P]
            eye_s = te_s[:, P:2 * P]

            # ---- input DMAs, demand-ordered with ENFORCED flow control.
            # All rings share one ~280GB/s per-core DMA pool (chip-shared
            # engines), and the Tile scheduler reorders same-engine
            # instructions by readiness, so program order enforces nothing.
            # Each gated DMA's destination corner is pre-written by a tiny
            # DMA reading an earlier tile: the gated DMA inherits a WAW dep
            # on the gate, which inherits a RAW dep on the earlier
            # transfer.  hh=0 segment tiles stream b/t-split on the
            # scalar/sync rings at depth ~3; hh=1 tiles stream whole on
            # gpsimd+sync chained behind them.
            HB = P // 2

            def gate(eng, dst_corner, src_corner):
                eng.dma_start(out=dst_corner, in_=src_corner)

            def cor_b(pp, hh, ss):
                return xt_s[(pp, hh, ss)][0:1, 0:1, 0:1]

            def cor_t(pp, hh, ss):
                return xt_s[(pp, hh, ss)][HB:HB + 1, 0:1, 0:1]

            def dma_b(pp, hh, ss):
                nc.scalar.dma_start(out=xt_s[(pp, hh, ss)][0:HB],
                                    in_=xt_d[hh, ss, pp, 0:HB])

            def dma_t(pp, hh, ss):
                nc.sync.dma_start(out=xt_s[(pp, hh, ss)][HB:P],
                                  in_=xt_d[hh, ss, pp, HB:P])

            # scalar ring: bottoms of hh=0 seg tiles, depth-3 then 4
            sc_seq = [(0, 0), (1, 0), (2, 0), (3, 0),
                      (0, 1), (1, 1), (2, 1), (3, 1)]
            for i, (pp, ss) in enumerate(sc_seq):
                if i >= 3:
                    pq, sq = sc_seq[max(0, i - 4)]
                    gate(nc.scalar, cor_b(pp, 0, ss), cor_b(pq, 0, sq))
                dma_b(pp, 0, ss)
            # sync ring: w3 chunk 0 + tops of hh=0 tiles, depth-2->3,
            # then hh=1 whole tiles for pp=1,3 (overlapping hh=0 seg1)
            nc.sync.dma_start(out=w_s[:, 0:1], in_=w3_d[:, 0:1])
            dma_t(0, 0, 0)
            dma_t(1, 0, 0)
            for i, (pp, ss) in enumerate(sc_seq[2:], start=2):
                pq, sq = sc_seq[i - 2 if i < 4 else i - 3]
                gate(nc.sync, cor_t(pp, 0, ss), cor_t(pq, 0, sq))
                dma_t(pp, 0, ss)
            for i, (pp, ss) in enumerate([(1, 0), (3, 0), (1, 1), (3, 1)]):
                if i < 2:
                    pq, sq = [(0, 1), (2, 1)][i]
                    gate(nc.sync, cor_b(pp, 1, ss), cor_t(pq, 0, sq))
                else:
                    pq, sq = [(1, 0), (3, 0)][i - 2]
                    gate(nc.sync, cor_b(pp, 1, ss), cor_b(pq, 1, sq))
                nc.sync.dma_start(out=xt_s[(pp, 1, ss)][:],
                                  in_=xt_d[1, ss, pp])
            # gpsimd ring: rest of w3 (chunks 1-3 free, 4-7 gated) + consts
            # + hh=1 whole tiles pp=0,2 (overlapping hh=0 seg1)
            nc.gpsimd.dma_start(out=w_s[:, 1:nchunk // 2],
                                in_=w3_d[:, 1:nchunk // 2])
            nc.gpsimd.dma_start(out=b_s[:], in_=b3_d[:])
            nc.gpsimd.dma_start(out=te_s[:], in_=te_d[:])
            nc.gpsimd.dma_start(out=w_s[:, 4:6], in_=w3_d[:, 4:6])
            gate(nc.gpsimd, w_s[0:1, 6:7, 0:1, 0:1], cor_b(0, 0, 0))
            nc.gpsimd.dma_start(out=w_s[:, 6:8], in_=w3_d[:, 6:8])
            for i, (pp, ss) in enumerate([(0, 0), (2, 0), (0, 1), (2, 1)]):
                if i < 2:
                    pq, sq = [(0, 1), (2, 1)][i]
                    gate(nc.gpsimd, cor_b(pp, 1, ss), cor_b(pq, 0, sq))
                else:
                    pq, sq = [(0, 0), (2, 0)][i - 2]
                    gate(nc.gpsimd, cor_b(pp, 1, ss), cor_b(pq, 1, sq))
                nc.gpsimd.dma_start(out=xt_s[(pp, 1, ss)][:],
                                    in_=xt_d[1, ss, pp])

            qt_s = bpool.tile([P, t], BF16, tag="qt", name="qt_t")
            kt_s = bpool.tile([P, t], BF16, tag="kt", name="kt_t")
            vt_s = bpool.tile([P, t], BF16, tag="vt", name="vt_t")
            v_s = [None] * ntile
            W = {"wq": 0, "wk": 1, "wv": 2}
            NAMES = ("wq", "wk", "wv")
            DST = {"wq": qt_s, "wk": kt_s, "wv": vt_s}

            acc = {}

            def proj_open(hf, s2):
                for name in NAMES:
                    acc[name] = pspool.tile([P, SEG], F32, tag="acc", bufs=3,
                                            name=f"acc_{name}_{hf}_{s2}")

            def proj_mms(hf, s2, cc_list):
                for cc in cc_list:
                    for name in NAMES:
                        nc.tensor.matmul(
                            acc[name][:],
                            w_s[:, cc, W[name], :],
                            xt_s[(cc // 2, hf, s2)][:, cc % 2, :],
                            start=(cc == 0), stop=(cc == nchunk - 1),
                        )

            def proj_drain(hf, s2):
                base = hf * hw + s2 * SEG
                for name in ("wk", "wq", "wv"):  # k first (gates S), q, v
                    nc.vector.tensor_scalar_add(
                        DST[name][:, base:base + SEG], acc[name][:],
                        b_s[:, W[name]:W[name] + 1])

            # all V tiles allocated up front; ones column memset once on
            # gpsimd so vtrans only needs the PSUM->SBUF copy
            for m in range(ntile):
                v_s[m] = vpool.tile([P, H + 1], BF16, tag="v",
                                    name=f"vtile{m}")
                nc.gpsimd.memset(v_s[m][:, H:H + 1], 1.0)
            Copy = mybir.ActivationFunctionType.Copy

            def vtrans2(m0, tail=False):
                # two transposes share one 2-slot PSUM tile so they don't
                # serialize on the single pso bank; the PSUM->SBUF copies go
                # to ACT (mid-kernel) so DVE drains aren't delayed
                pst = pspool.tile([P, 2, P], BF16, tag="pso", bufs=2,
                                  name=f"pst{m0}")
                for u in (0, 1):
                    m = m0 + u
                    nc.tensor.transpose(pst[:, u],
                                        vt_s[:, m * P:(m + 1) * P], eye_s)
                    if tail:
                        nc.vector.tensor_copy(v_s[m][:, 0:H], pst[:, u])
                    else:
                        nc.scalar.activation(v_s[m][:, 0:H], pst[:, u], Copy)

            es_all = {}

            def S_unit(j, m):
                es_of = es_all.setdefault(j, [None] * (tpb * j + tpb))
                r = m - tpb * j
                off = P * r if r > 0 else 0
                ps = pspool.tile([P, SEG], F32, tag="spsum", bufs=3,
                                 name=f"sps{j}_{m}")
                es = espool.tile([P, SEG], BF16, tag="es", name=f"es{j}_{m}")
                nc.tensor.matmul(
                    ps[:, off:SEG],
                    kt_s[:, m * P:(m + 1) * P],
                    qt_s[:, j * SEG + off:(j + 1) * SEG],
                    start=True, stop=True,
                )
                nc.scalar.activation(es[:, off:SEG], ps[:, off:SEG], Exp,
                                     scale=SCALE)
                if r >= 0:
                    # diagonal tile: triangle mask on the [128,128] block
                    nc.gpsimd.tensor_mul(
                        es[:, off:off + P], es[:, off:off + P], tri_s)
                es_of[m] = es

            pts = [None, None, None]
            chain_no = [0]

            def pv_open():
                # PV accumulators reuse the (now dead) proj-acc banks:
                # 3 tiles x 2 slots = 6 chains in flight
                for x in range(3):
                    pts[x] = pspool.tile([P, 2, H + 32], F32, tag="acc",
                                         bufs=3, name=f"pv_ps{x}")

            obs = {}

            def pv_ob(j):
                obs[j] = opool.tile([P, tpb, H], F32, tag="ob", bufs=2,
                                    name=f"ob{j}")

            def PV_chain(j, rr):
                es_of = es_all[j]
                ob = obs[j]
                i = tpb * j + rr
                cn = chain_no[0]
                chain_no[0] += 1
                pso = pts[cn % 3][:, (cn // 3) % 2, 0:H + 1]
                for m in range(i + 1):
                    nc.tensor.matmul(
                        pso[:],
                        es_of[m][:, rr * P:rr * P + P],
                        v_s[m][:],
                        start=(m == 0), stop=(m == i),
                    )
                rc = opool.tile([P, 1], F32, tag="rc", bufs=4, name=f"rc{i}")
                nc.vector.reciprocal(rc[:], pso[:, H:H + 1])
                nc.vector.tensor_scalar_mul(ob[:, rr, :], pso[:, 0:H], rc[:])
                q = (nc.sync, nc.gpsimd, nc.scalar)[i % 3]
                q.dma_start(out=out_d[i * P:(i + 1) * P, :], in_=ob[:, rr, :])

            def fill(n):
                for _ in range(n):
                    nc.tensor.matmul(warm_ps[:, 1, 0:P], warm_s[:, 0:P],
                                     warm_s[:, P:2 * P], start=True,
                                     stop=True)

            if t >= 2048:
                # fillers woven through the DMA-paced proj0 era keep the
                # HAM activity window busy (no mid-kernel re-throttle)
                proj_open(0, 0)
                for cc in range(nchunk):
                    proj_mms(0, 0, [cc])
                    fill(3)
                proj_drain(0, 0)
                fill(2)
                S_unit(0, 0); S_unit(0, 1)
                proj_open(0, 1)
                proj_mms(0, 1, range(0, 2)); fill(2)
                vtrans2(0)
                proj_mms(0, 1, range(2, 4)); fill(2)
                S_unit(0, 2); S_unit(0, 3)
                vtrans2(2)
                proj_mms(0, 1, range(4, 6)); fill(2)
                proj_mms(0, 1, range(6, 8))
                proj_drain(0, 1)
                fill(2)
                S_unit(1, 0); S_unit(1, 1)
                proj_open(1, 0)
                proj_mms(1, 0, range(0, 2))
                S_unit(1, 2); S_unit(1, 3)
                proj_mms(1, 0, range(2, 4))
                vtrans2(4)
                S_unit(1, 4); S_unit(1, 5)
                proj_mms(1, 0, range(4, 6))
                vtrans2(6)
                proj_mms(1, 0, range(6, 8))
                proj_drain(1, 0)
                S_unit(1, 6); S_unit(1, 7)
                proj_open(1, 1)
                proj_mms(1, 1, range(0, 2))
                S_unit(2, 0); S_unit(2, 1)
                proj_mms(1, 1, range(2, 4))
                S_unit(2, 2); S_unit(2, 3)
                vtrans2(8)
                proj_mms(1, 1, range(4, 6))
                S_unit(2, 4); S_unit(2, 5)
                vtrans2(10)
                proj_mms(1, 1, range(6, 8))
                S_unit(2, 6); S_unit(2, 7)
                proj_drain(1, 1)
                # ---- tail: S2 diag + S3 (diag first) woven with all PV
                S_unit(2, 8); S_unit(2, 9)
                pv_open()
                pv_ob(0); pv_ob(1)
                PV_chain(0, 0)
                S_unit(2, 10)
                PV_chain(0, 1)
                S_unit(2, 11)
                PV_chain(0, 2)
                S_unit(3, 12)
                PV_chain(0, 3)
                S_unit(3, 13)
                PV_chain(1, 0)
                S_unit(3, 14)
                PV_chain(1, 1)
                S_unit(3, 15)
                PV_chain(1, 2)
                S_unit(3, 0)
                PV_chain(1, 3)
                S_unit(3, 1)
                vtrans2(12, tail=True)
                S_unit(3, 2)
                vtrans2(14, tail=True)
                S_unit(3, 3)
                pv_ob(2)
                PV_chain(2, 0)
                S_unit(3, 4)
                PV_chain(2, 1)
                S_unit(3, 8)
                S_unit(3, 5)
                S_unit(3, 9)
                S_unit(3, 6)
                S_unit(3, 10)
                S_unit(3, 7)
                S_unit(3, 11)
                PV_chain(2, 2)
                PV_chain(2, 3)
                pv_ob(3)
                PV_chain(3, 0)
                PV_chain(3, 1)
                PV_chain(3, 2)
                PV_chain(3, 3)
            else:
                for hf in range(nhalf):
                    for s2 in range(hw // SEG):
                        proj_open(hf, s2)
                        proj_mms(hf, s2, range(nchunk))
                        proj_drain(hf, s2)
                for m0 in range(0, ntile, 2):
                    vtrans2(m0)
                pv_open()
                for j in range(nblk):
                    for m in range(tpb * j + tpb):
                        S_unit(j, m)
                    pv_ob(j)
                    for rr in range(tpb):
                        PV_chain(j, rr)

    nc.finalize()
    return nc


_NC_CACHE = {}


def _get_nc(t=T, reps=1):
    key = (t, reps)
    if key not in _NC_CACHE:
        _NC_CACHE[key] = build_nc(t, reps)
    return _NC_CACHE[key]


def make_in_maps(embedded_data, Wq, bq, Wk, bk, Wv, bv, t=T):
    bf = ml_dtypes.bfloat16
    tri = np.triu(np.ones((P, P), dtype=np.float32))  # tri[k,q]=1 iff q>=k
    eye = np.eye(P, dtype=np.float32)
    te = np.concatenate([tri, eye], axis=1).astype(bf)
    w3 = np.stack([np.asarray(w, np.float32) for w in (Wq, Wk, Wv)])  # [3,C,H]
    # pre-transpose to [P, C//P, 3, H] so the DMA is contiguous per partition
    w3 = np.ascontiguousarray(
        w3.reshape(3, C // P, P, H).transpose(2, 1, 0, 3)).astype(bf)
    b3 = np.stack(
        [np.asarray(x, np.float32).reshape(H) for x in (bq, bk, bv)], axis=1)
    shared = {"w3": w3, "b3": np.ascontiguousarray(b3), "te": te}
    nhalf = max(1, t // 1024)
    hw = t // nhalf
    nseg = hw // 512
    in_maps = []
    for b in range(NCORES):
        m = dict(shared)
        xtf = np.asarray(embedded_data[b], np.float32).T[:, :t]  # [C, t]
        # [pp, e, p, hh, ss, col] -> [hh, ss, pp, p, e, col]
        arr = xtf.reshape(C // P // 2, 2, P, nhalf, nseg, 512)
        arr = arr.transpose(3, 4, 0, 2, 1, 5)
        m["xt"] = np.ascontiguousarray(arr).astype(bf)
        in_maps.append(m)
    return in_maps


def kernel(embedded_data, Wq, bq, Wk, bk, Wv, bv, trace=False):
    global LAST_RESULT
    nc = _get_nc(T)
    in_maps = make_in_maps(embedded_data, Wq, bq, Wk, bk, Wv, bv, T)
    res = run_bass_kernel_spmd(nc, in_maps, core_ids=list(range(NCORES)), trace=trace)
    LAST_RESULT = res
    out = np.stack([np.asarray(res.results[i]["out"]) for i in range(NCORES)])
    return out.astype(np.float32)
